# revision 18
# baseline (speedup 1.0000x reference)
"""Multi-head causal attention (B=2, S=2048, HID=2048, H=16, D=128) on 8 TRN2
NeuronCores.

Sharding: core c handles batch b=c//4 and heads [4*(c%4) .. 4*(c%4)+3].
Each core computes qkv-projection + RoPE + causal attention + its partial
out-projection; the host sums the 4 partial outputs per batch (tensor-parallel
reduce) and stacks the 2 batches.

v3: all matmul operands bf16 (fast weight load, half DMA), x/weights DMA'd
once (fused V+QK projection), DMA issue order puts the first matmul's
dependencies first, per-jb qkT tiles and per-chunk V tiles for fine-grained
dependencies, attention software-pipelined one k-pair ahead, diagonal k-chunks
trimmed to the causally-live q-range, fast-approx softmax reciprocal, bf16
partial output.

On-chip layout: activations kept transposed ([feature, token]):
  qT/kT = W_qk^T-slice @ x^T   (RoPE applied during PSUM evacuation)
  S^T[k,q] = kT^T@qT ; A = exp(S^T*scale) (*causal mask on the diagonal)
  outT[d,q] = V^T-chunks @ A   (accumulated over k chunks)
  y[tok,col] = outT^T-chunks @ W_o-rows  (accumulated over heads)
Softmax row-sums come from a ones[128,128] matmul in the [k,q] layout.
"""
import sys

sys.path.insert(0, '/opt/trn_rl_repo')

import numpy as np
import ml_dtypes

B, S, HID = 2, 2048, 2048
H, D = 16, 128
NH = H // 4          # heads per core = 4
HC = HID // 128      # hid chunks = 16
TB = 512             # token block for projection
NTB = S // TB        # 4
QB = 512             # q block in attention
NQB = S // QB        # 4
NKCH = S // 128      # k chunks total = 16
SCALE = 1.0 / float(np.sqrt(D))
BASE = 10000.0
N_CORES = 8

_cache = {}


def _build():
    import concourse.bass as bass  # noqa: F401
    import concourse.tile as tile
    from concourse import bacc, mybir

    f32 = mybir.dt.float32
    bf16 = mybir.dt.bfloat16
    EXP = mybir.ActivationFunctionType.Exp
    MULT = mybir.AluOpType.mult
    ADD = mybir.AluOpType.add

    nc = bacc.Bacc("TRN2", target_bir_lowering=False, debug=False,
                   num_devices=N_CORES)

    xT = nc.dram_tensor("xT", [HID, S], bf16, kind="ExternalInput").ap()
    wqk = nc.dram_tensor("wqk", [HID, 2 * NH * D], bf16, kind="ExternalInput").ap()
    wv = nc.dram_tensor("wv", [HID, NH * D], bf16, kind="ExternalInput").ap()
    wo = nc.dram_tensor("wo", [NH * D, HID], bf16, kind="ExternalInput").ap()
    cosT = nc.dram_tensor("cosT", [D, S], f32, kind="ExternalInput").ap()
    sinS = nc.dram_tensor("sinS", [D, S], f32, kind="ExternalInput").ap()
    # tri[k, q] = (q >= k) for the 128x128 leading triangle of a diag chunk
    triM = nc.dram_tensor("triM", [128, 128], bf16, kind="ExternalInput").ap()
    ones_sq = nc.dram_tensor("ones_sq", [128, 128], bf16, kind="ExternalInput").ap()
    y = nc.dram_tensor("y", [S, HID], bf16, kind="ExternalOutput").ap()
    warm = nc.dram_tensor("warm", [128, 8], f32, kind="ExternalOutput").ap()

    with tile.TileContext(nc) as tc:
        with tc.tile_pool(name="persist", bufs=1) as pp:
            # per-(col, jb) qk tiles and per-chunk v tiles → fine-grained deps
            qkT = [[pp.tile([128, TB], bf16, tag=f"qkT{i}_{j}",
                            name=f"qkT{i}_{j}") for j in range(NTB)]
                   for i in range(8)]
            v_t = [pp.tile([128, NH * D], bf16, tag=f"v{cg}", name=f"v{cg}")
                   for cg in range(NKCH)]

            # ---- phase 1: fused V + QK projection (x and weights loaded once)
            with tc.tile_pool(name="p1w", bufs=1) as p1w, \
                 tc.tile_pool(name="p1x", bufs=2) as p1x, \
                 tc.tile_pool(name="rope", bufs=2) as rp, \
                 tc.tile_pool(name="trig", bufs=1) as tp, \
                 tc.tile_pool(name="psQK", bufs=2, space="PSUM") as psq, \
                 tc.tile_pool(name="psV", bufs=2, space="PSUM") as psv:
                # PE warmup: ones lands in ~1us; a burst of dummy matmuls
                # holds the PE busy through the DMA-bound start so the HAM
                # clock-gate opens to 8/8 before the real matmuls issue
                wones = p1w.tile([128, 128], bf16, tag="wones")
                nc.sync.dma_start(wones[:], ones_sq[:])
                Pw = psv.tile([128, 128], f32, tag="Pw", name="Pw")
                for _ in range(64):
                    nc.tensor.matmul(Pw[:], wones[:], wones[:],
                                     start=True, stop=True)
                wsb = p1w.tile([128, 8], f32, tag="wsb")
                nc.scalar.copy(wsb[:], Pw[:, 0:8])
                nc.sync.dma_start(warm, wsb[:])

                # per-chunk weight and x tiles: the first matmuls can start
                # as soon as their own 128-row chunk has landed
                wv_c = [p1w.tile([128, NH * D], bf16, tag=f"wv{c}",
                                 name=f"wv{c}") for c in range(HC)]
                wq_c = [p1w.tile([128, 2 * NH * D], bf16, tag=f"wq{c}",
                                 name=f"wq{c}") for c in range(HC)]
                xc_t = [[p1x.tile([128, TB], bf16, tag=f"xc{c}",
                                  name=f"xc{jb}_{c}") for c in range(HC)]
                        for jb in range(NTB)]

                def dma_x(jb, c):
                    nc.sync.dma_start(
                        xc_t[jb][c][:],
                        xT[c * 128:(c + 1) * 128, jb * TB:(jb + 1) * TB])

                # DMA issue order = first-matmul dependencies first:
                # interleave wv / x(jb0) chunk-wise, then wqk, trig
                for c in range(HC):
                    nc.sync.dma_start(
                        wv_c[c][:], wv[c * 128:(c + 1) * 128, :])
                    dma_x(0, c)
                for c in range(HC):
                    nc.sync.dma_start(
                        wq_c[c][:], wqk[c * 128:(c + 1) * 128, :])
                tcos = tp.tile([D, S], f32, tag="tcos")
                tsin = tp.tile([D, S], f32, tag="tsin")
                nc.sync.dma_start(tcos[:], cosT)
                nc.sync.dma_start(tsin[:], sinS)

                for jb in range(NTB):
                    if jb + 1 < NTB:
                        for c in range(HC):
                            dma_x(jb + 1, c)  # prefetch next token block
                    xc = xc_t[jb]
                    last = jb == NTB - 1

                    def v_proj():
                        for t2 in range(TB // 128):
                            cg = jb * (TB // 128) + t2  # 128-token chunk
                            Pv = psv.tile([128, NH * D], f32, tag="Pv",
                                          name="Pv")
                            for c in range(HC):
                                nc.tensor.matmul(
                                    Pv[:],
                                    xc[c][:, t2 * 128:(t2 + 1) * 128],
                                    wv_c[c][:],
                                    start=(c == 0), stop=(c == HC - 1))
                            nc.scalar.copy(v_t[cg][:], Pv[:])

                    def qk_proj():
                        sl = slice(jb * TB, (jb + 1) * TB)
                        for cc in range(8):  # 4 q cols then 4 k cols
                            P = psq.tile([128, TB], f32, tag="P", name="P")
                            for c in range(HC):
                                nc.tensor.matmul(
                                    P[:],
                                    wq_c[c][:, cc * 128:(cc + 1) * 128],
                                    xc[c][:],
                                    start=(c == 0), stop=(c == HC - 1))
                            u = rp.tile([128, TB], f32, tag="u", name="u")
                            nc.scalar.copy(u[:], P[:])
                            rot = rp.tile([128, TB], f32, tag="rot", name="rot")
                            nc.sync.dma_start(rot[0:64, :], u[64:128, :])
                            nc.sync.dma_start(rot[64:128, :], u[0:64, :])
                            t = rp.tile([128, TB], f32, tag="t", name="t")
                            nc.vector.tensor_tensor(
                                out=t[:], in0=u[:], in1=tcos[:, sl], op=MULT)
                            m = rp.tile([128, TB], f32, tag="m", name="m")
                            nc.vector.tensor_tensor(
                                out=m[:], in0=rot[:], in1=tsin[:, sl], op=MULT)
                            nc.vector.tensor_tensor(
                                out=qkT[cc][jb][:], in0=t[:], in1=m[:], op=ADD)

                    # last block runs QK first so the rope tail drains behind
                    # the V matmuls instead of stalling the attention phase
                    if last:
                        qk_proj()
                        v_proj()
                    else:
                        v_proj()
                        qk_proj()

            # ---- phases 2+3 share the wot/outT pool ----
            with tc.tile_pool(name="p23w", bufs=1) as p2w:
              wot = [p2w.tile([128, HID], bf16, tag=f"wot{h}", name=f"wot{h}")
                     for h in range(NH)]
              for h in range(NH):
                  nc.sync.dma_start(wot[h][:], wo[h * 128:(h + 1) * 128, :])
              outT = [[p2w.tile([128, QB], bf16, tag=f"outT{h}_{j}",
                                name=f"outT{h}_{j}") for j in range(NQB)]
                      for h in range(NH)]

              # ---- phase 2: attention (software-pipelined two pairs ahead) ----
              with tc.tile_pool(name="p2", bufs=10) as p2, \
                 tc.tile_pool(name="p2c", bufs=1) as p2c, \
                 tc.tile_pool(name="p2r", bufs=2) as p2r, \
                 tc.tile_pool(name="psS", bufs=2, space="PSUM") as psS, \
                 tc.tile_pool(name="psO", bufs=2, space="PSUM") as psO, \
                 tc.tile_pool(name="psR", bufs=2, space="PSUM") as psR:
                tri = p2c.tile([128, 128], bf16, tag="tri")
                nc.sync.dma_start(tri[:], triM[:])
                t1s = p2c.tile([128, 128], bf16, tag="t1s")
                nc.sync.dma_start(t1s[:], ones_sq[:])

                # flat list of k-chunk-pair units across all (h, q-block)
                units = []
                for h in range(NH):
                    for jb4 in range(NQB):
                        nkc = (QB // 128) * (jb4 + 1)
                        for kp in range(nkc // 2):
                            units.append((h, jb4, kp, nkc))

                blocks = {}   # (h, jb4) -> dict with O, R, A-tiles

                def emit_scores(unit):
                    """S-pair matmuls + exp (+ diag mask) for one unit."""
                    h, jb4, kp, nkc = unit
                    kc0 = 2 * kp
                    kT = qkT[NH + h]
                    qT_b = qkT[h][jb4]
                    Sc = psS.tile([128, 2 * QB], f32, tag="S", name="S")
                    md = kc0 - (QB // 128) * jb4
                    if md < 0:  # fully below the diagonal: full pair
                        for i in range(2):
                            kc = kc0 + i
                            nc.tensor.matmul(
                                Sc[:, i * QB:(i + 1) * QB],
                                kT[kc // 4][:, (kc % 4) * 128:(kc % 4 + 1) * 128],
                                qT_b[:], start=True, stop=True)
                        A = p2.tile([128, 2 * QB], bf16, tag="A", name="A")
                        nc.scalar.activation(A[:], Sc[:], EXP, scale=SCALE)
                    else:  # diagonal pair: trim to live q-range per chunk
                        A = p2.tile([128, 2 * QB], bf16, tag="A", name="A")
                        for i in range(2):
                            kc = kc0 + i
                            off = (md + i) * 128
                            nc.tensor.matmul(
                                Sc[:, i * QB + off:(i + 1) * QB],
                                kT[kc // 4][:, (kc % 4) * 128:(kc % 4 + 1) * 128],
                                qT_b[:, off:], start=True, stop=True)
                            nc.scalar.activation(
                                A[:, i * QB + off:(i + 1) * QB],
                                Sc[:, i * QB + off:(i + 1) * QB],
                                EXP, scale=SCALE)
                            # causal triangle on the leading 128 q columns
                            nc.vector.tensor_tensor(
                                out=A[:, i * QB + off:i * QB + off + 128],
                                in0=A[:, i * QB + off:i * QB + off + 128],
                                in1=tri[:], op=MULT)
                    blocks[(h, jb4)].setdefault("A", {})[kp] = A

                def emit_av(unit):
                    """AV matmuls for one unit (two pairs behind); the rowsum
                    matmuls batch at block end — by then they have no pending
                    exp dependency, so they hide the next block's ramp."""
                    h, jb4, kp, nkc = unit
                    blk = blocks[(h, jb4)]
                    if kp == 0:  # allocate accumulators at first use
                        blk["O"] = psO.tile([128, QB], f32, tag="O", name="O")
                        blk["R"] = psR.tile([128, QB], f32, tag="R", name="R")
                    A = blk["A"][kp]
                    kc0 = 2 * kp
                    md = kc0 - (QB // 128) * jb4
                    for i in range(2):
                        kc = kc0 + i
                        off = max(0, (md + i) * 128)
                        nc.tensor.matmul(
                            blk["O"][:, off:],
                            v_t[kc][:, h * D:(h + 1) * D],
                            A[:, i * QB + off:(i + 1) * QB],
                            start=(kc == 0), stop=(kc == nkc - 1),
                            skip_group_check=True)
                    if kp == nkc // 2 - 1:  # block finished: rowsums, norm
                        for kp2 in range(nkc // 2):
                            A2 = blk["A"].pop(kp2)
                            md2 = 2 * kp2 - (QB // 128) * jb4
                            for i in range(2):
                                kc = 2 * kp2 + i
                                off = max(0, (md2 + i) * 128)
                                nc.tensor.matmul(
                                    blk["R"][:, off:], t1s[:],
                                    A2[:, i * QB + off:(i + 1) * QB],
                                    start=(kc == 0), stop=(kc == nkc - 1),
                                    skip_group_check=True)
                        rc = p2r.tile([128, QB], f32, tag="rc", name="rc")
                        nc.vector.reciprocal_approx_fast(rc[:], blk["R"][:])
                        nc.vector.tensor_tensor(
                            out=outT[h][jb4][:], in0=blk["O"][:], in1=rc[:],
                            op=MULT)

                DEPTH = 2
                for idx, unit in enumerate(units):
                    h, jb4, kp, nkc = unit
                    if kp == 0:
                        blocks[(h, jb4)] = {}
                    emit_scores(unit)
                    if idx >= DEPTH:
                        emit_av(units[idx - DEPTH])
                for idx in range(len(units) - DEPTH, len(units)):
                    emit_av(units[idx])

              # ---- phase 3: out projection (partial) ----
              with tc.tile_pool(name="p3", bufs=4) as p3, \
                   tc.tile_pool(name="ps3", bufs=4, space="PSUM") as ps3:
                  for tch in range(S // 128):
                      for cbp in range(HID // 1024):
                          ys = p3.tile([128, 1024], bf16, tag="ys", name="ys")
                          for cb2 in range(2):
                              cb = cbp * 2 + cb2
                              P3 = ps3.tile([128, 512], f32, tag="P3", name="P3")
                              for h in range(NH):
                                  nc.tensor.matmul(
                                      P3[:],
                                      outT[h][tch // 4][
                                          :, (tch % 4) * 128:
                                          (tch % 4 + 1) * 128],
                                      wot[h][:, cb * 512:(cb + 1) * 512],
                                      start=(h == 0), stop=(h == NH - 1))
                              dst = ys[:, cb2 * 512:(cb2 + 1) * 512]
                              if cb % 2 == 0:
                                  nc.vector.tensor_copy(dst, P3[:])
                              else:
                                  nc.scalar.copy(dst, P3[:])
                          nc.sync.dma_start(
                              y[tch * 128:(tch + 1) * 128,
                                cbp * 1024:(cbp + 1) * 1024], ys[:])

    nc.compile()
    return nc


def _host_inputs(x, w_qkv, w_out):
    """Build the 8 per-core input maps."""
    bf16 = ml_dtypes.bfloat16
    # RoPE tables, transposed ([d, t]) with the rotate-half sign folded in.
    inv_freq = 1.0 / (BASE ** (np.arange(0, D, 2, dtype=np.float64) / D))
    pos = np.arange(S, dtype=np.float64)
    freqs = np.outer(inv_freq, pos)           # [64, S]
    cos_h = np.cos(freqs).astype(np.float32)
    sin_h = np.sin(freqs).astype(np.float32)
    cosT = np.concatenate([cos_h, cos_h], 0)  # [128, S]
    sinS = np.concatenate([-sin_h, sin_h], 0)  # signed sin

    # 128x128 causal triangle ([k-part, q-free]): live where q >= k
    kp = np.arange(128)[:, None]
    qf = np.arange(128)[None, :]
    triM = (qf >= kp).astype(bf16)

    w3 = np.asarray(w_qkv, np.float32).reshape(HID, 3, H, D)
    wo_full = np.asarray(w_out, np.float32).reshape(H, D, HID)
    x = np.asarray(x, np.float32)
    xT_b = [np.ascontiguousarray(x[b].T).astype(bf16) for b in range(B)]

    shared = {
        "cosT": cosT, "sinS": sinS, "triM": triM,
        "ones_sq": np.ones((128, 128), bf16),
    }
    in_maps = []
    for c in range(N_CORES):
        b, hg = c // 4, c % 4
        heads = slice(4 * hg, 4 * hg + 4)
        wqk = np.ascontiguousarray(
            w3[:, 0:2, heads, :].reshape(HID, 2 * NH * D)).astype(bf16)
        wv = np.ascontiguousarray(
            w3[:, 2, heads, :].reshape(HID, NH * D)).astype(bf16)
        wo_c = np.ascontiguousarray(
            wo_full[heads].reshape(NH * D, HID)).astype(bf16)
        in_maps.append({
            "xT": xT_b[b],
            "wqk": wqk, "wv": wv, "wo": wo_c, **shared,
        })
    return in_maps


def kernel(x, w_qkv, w_out):
    from concourse.bass_utils import run_bass_kernel_spmd

    if "nc" not in _cache:
        _cache["nc"] = _build()
    nc = _cache["nc"]
    in_maps = _host_inputs(x, w_qkv, w_out)
    res = run_bass_kernel_spmd(nc, in_maps, core_ids=list(range(N_CORES)))
    out = np.zeros((B, S, HID), np.float32)
    for c in range(N_CORES):
        out[c // 4] += res.results[c]["y"].astype(np.float32)
    return out


# revision 19
# speedup vs baseline: 1.0014x; 1.0014x over previous
"""Multi-head causal attention (B=2, S=2048, HID=2048, H=16, D=128) on 8 TRN2
NeuronCores.

Sharding: core c handles batch b=c//4 and heads [4*(c%4) .. 4*(c%4)+3].
Each core computes qkv-projection + RoPE + causal attention + its partial
out-projection; the host sums the 4 partial outputs per batch (tensor-parallel
reduce) and stacks the 2 batches.

v3: all matmul operands bf16 (fast weight load, half DMA), x/weights DMA'd
once (fused V+QK projection), DMA issue order puts the first matmul's
dependencies first, per-jb qkT tiles and per-chunk V tiles for fine-grained
dependencies, attention software-pipelined one k-pair ahead, diagonal k-chunks
trimmed to the causally-live q-range, fast-approx softmax reciprocal, bf16
partial output.

On-chip layout: activations kept transposed ([feature, token]):
  qT/kT = W_qk^T-slice @ x^T   (RoPE applied during PSUM evacuation)
  S^T[k,q] = kT^T@qT ; A = exp(S^T*scale) (*causal mask on the diagonal)
  outT[d,q] = V^T-chunks @ A   (accumulated over k chunks)
  y[tok,col] = outT^T-chunks @ W_o-rows  (accumulated over heads)
Softmax row-sums come from a ones[128,128] matmul in the [k,q] layout.
"""
import sys

sys.path.insert(0, '/opt/trn_rl_repo')

import numpy as np
import ml_dtypes

B, S, HID = 2, 2048, 2048
H, D = 16, 128
NH = H // 4          # heads per core = 4
HC = HID // 128      # hid chunks = 16
TB = 512             # token block for projection
NTB = S // TB        # 4
QB = 512             # q block in attention
NQB = S // QB        # 4
NKCH = S // 128      # k chunks total = 16
SCALE = 1.0 / float(np.sqrt(D))
BASE = 10000.0
N_CORES = 8

_cache = {}


def _build():
    import concourse.bass as bass  # noqa: F401
    import concourse.tile as tile
    from concourse import bacc, mybir

    f32 = mybir.dt.float32
    bf16 = mybir.dt.bfloat16
    EXP = mybir.ActivationFunctionType.Exp
    MULT = mybir.AluOpType.mult
    ADD = mybir.AluOpType.add

    nc = bacc.Bacc("TRN2", target_bir_lowering=False, debug=False,
                   num_devices=N_CORES)

    xT = nc.dram_tensor("xT", [HID, S], bf16, kind="ExternalInput").ap()
    wqk = nc.dram_tensor("wqk", [HID, 2 * NH * D], bf16, kind="ExternalInput").ap()
    wv = nc.dram_tensor("wv", [HID, NH * D], bf16, kind="ExternalInput").ap()
    wo = nc.dram_tensor("wo", [NH * D, HID], bf16, kind="ExternalInput").ap()
    cosT = nc.dram_tensor("cosT", [D, S], f32, kind="ExternalInput").ap()
    sinS = nc.dram_tensor("sinS", [D, S], f32, kind="ExternalInput").ap()
    # tri[k, q] = (q >= k) for the 128x128 leading triangle of a diag chunk
    triM = nc.dram_tensor("triM", [128, 128], bf16, kind="ExternalInput").ap()
    ones_sq = nc.dram_tensor("ones_sq", [128, 128], bf16, kind="ExternalInput").ap()
    y = nc.dram_tensor("y", [S, HID], bf16, kind="ExternalOutput").ap()
    warm = nc.dram_tensor("warm", [128, 8], f32, kind="ExternalOutput").ap()

    with tile.TileContext(nc) as tc:
        with tc.tile_pool(name="persist", bufs=1) as pp:
            # per-(col, jb) qk tiles and per-chunk v tiles → fine-grained deps
            qkT = [[pp.tile([128, TB], bf16, tag=f"qkT{i}_{j}",
                            name=f"qkT{i}_{j}") for j in range(NTB)]
                   for i in range(8)]
            v_t = [pp.tile([128, NH * D], bf16, tag=f"v{cg}", name=f"v{cg}")
                   for cg in range(NKCH)]

            # ---- phase 1: fused V + QK projection (x and weights loaded once)
            with tc.tile_pool(name="p1w", bufs=1) as p1w, \
                 tc.tile_pool(name="p1x", bufs=2) as p1x, \
                 tc.tile_pool(name="rope", bufs=2) as rp, \
                 tc.tile_pool(name="trig", bufs=1) as tp, \
                 tc.tile_pool(name="psQK", bufs=2, space="PSUM") as psq, \
                 tc.tile_pool(name="psV", bufs=2, space="PSUM") as psv:
                # PE warmup: ones lands in ~1us; a burst of dummy matmuls
                # holds the PE busy through the DMA-bound start so the HAM
                # clock-gate opens to 8/8 before the real matmuls issue
                wones = p1w.tile([128, 128], bf16, tag="wones")
                nc.sync.dma_start(wones[:], ones_sq[:])
                Pw = psv.tile([128, 128], f32, tag="Pw", name="Pw")
                for _ in range(64):
                    nc.tensor.matmul(Pw[:], wones[:], wones[:],
                                     start=True, stop=True)
                wsb = p1w.tile([128, 8], f32, tag="wsb")
                nc.scalar.copy(wsb[:], Pw[:, 0:8])
                nc.sync.dma_start(warm, wsb[:])

                # per-chunk weight and x tiles: the first matmuls can start
                # as soon as their own 128-row chunk has landed
                wv_c = [p1w.tile([128, NH * D], bf16, tag=f"wv{c}",
                                 name=f"wv{c}") for c in range(HC)]
                wq_c = [p1w.tile([128, 2 * NH * D], bf16, tag=f"wq{c}",
                                 name=f"wq{c}") for c in range(HC)]
                xc_t = [[p1x.tile([128, TB], bf16, tag=f"xc{c}",
                                  name=f"xc{jb}_{c}") for c in range(HC)]
                        for jb in range(NTB)]

                def dma_x(jb, c):
                    nc.sync.dma_start(
                        xc_t[jb][c][:],
                        xT[c * 128:(c + 1) * 128, jb * TB:(jb + 1) * TB])

                # DMA issue order = first-matmul dependencies first:
                # interleave wv / x(jb0) chunk-wise, then wqk, trig
                for c in range(HC):
                    nc.sync.dma_start(
                        wv_c[c][:], wv[c * 128:(c + 1) * 128, :])
                    dma_x(0, c)
                for c in range(HC):
                    nc.sync.dma_start(
                        wq_c[c][:], wqk[c * 128:(c + 1) * 128, :])
                tcos = tp.tile([D, S], f32, tag="tcos")
                tsin = tp.tile([D, S], f32, tag="tsin")
                nc.sync.dma_start(tcos[:], cosT)
                nc.sync.dma_start(tsin[:], sinS)

                for jb in range(NTB):
                    if jb + 1 < NTB:
                        for c in range(HC):
                            dma_x(jb + 1, c)  # prefetch next token block
                    xc = xc_t[jb]
                    last = jb == NTB - 1

                    def v_proj():
                        for t2 in range(TB // 128):
                            cg = jb * (TB // 128) + t2  # 128-token chunk
                            Pv = psv.tile([128, NH * D], f32, tag="Pv",
                                          name="Pv")
                            for c in range(HC):
                                nc.tensor.matmul(
                                    Pv[:],
                                    xc[c][:, t2 * 128:(t2 + 1) * 128],
                                    wv_c[c][:],
                                    start=(c == 0), stop=(c == HC - 1))
                            nc.scalar.copy(v_t[cg][:], Pv[:])

                    def qk_proj():
                        sl = slice(jb * TB, (jb + 1) * TB)
                        for cc in range(8):  # 4 q cols then 4 k cols
                            P = psq.tile([128, TB], f32, tag="P", name="P")
                            for c in range(HC):
                                nc.tensor.matmul(
                                    P[:],
                                    wq_c[c][:, cc * 128:(cc + 1) * 128],
                                    xc[c][:],
                                    start=(c == 0), stop=(c == HC - 1))
                            u = rp.tile([128, TB], f32, tag="u", name="u")
                            nc.scalar.copy(u[:], P[:])
                            rot = rp.tile([128, TB], f32, tag="rot", name="rot")
                            nc.sync.dma_start(rot[0:64, :], u[64:128, :])
                            nc.sync.dma_start(rot[64:128, :], u[0:64, :])
                            t = rp.tile([128, TB], f32, tag="t", name="t")
                            nc.vector.tensor_tensor(
                                out=t[:], in0=u[:], in1=tcos[:, sl], op=MULT)
                            m = rp.tile([128, TB], f32, tag="m", name="m")
                            nc.vector.tensor_tensor(
                                out=m[:], in0=rot[:], in1=tsin[:, sl], op=MULT)
                            nc.vector.tensor_tensor(
                                out=qkT[cc][jb][:], in0=t[:], in1=m[:], op=ADD)

                    # last block runs QK first so the rope tail drains behind
                    # the V matmuls instead of stalling the attention phase
                    if last:
                        qk_proj()
                        v_proj()
                    else:
                        v_proj()
                        qk_proj()

            # ---- phases 2+3 share the wot/outT pool ----
            with tc.tile_pool(name="p23w", bufs=1) as p2w:
              wot = [p2w.tile([128, HID], bf16, tag=f"wot{h}", name=f"wot{h}")
                     for h in range(NH)]
              for h in range(NH):
                  nc.sync.dma_start(wot[h][:], wo[h * 128:(h + 1) * 128, :])
              outT = [p2w.tile([128, S], bf16, tag=f"outT{h}", name=f"outT{h}")
                      for h in range(NH)]

              # ---- phase 2: attention (software-pipelined two pairs ahead) ----
              with tc.tile_pool(name="p2", bufs=10) as p2, \
                 tc.tile_pool(name="p2c", bufs=1) as p2c, \
                 tc.tile_pool(name="p2r", bufs=2) as p2r, \
                 tc.tile_pool(name="psS", bufs=2, space="PSUM") as psS, \
                 tc.tile_pool(name="psO", bufs=2, space="PSUM") as psO, \
                 tc.tile_pool(name="psR", bufs=2, space="PSUM") as psR:
                tri = p2c.tile([128, 128], bf16, tag="tri")
                nc.sync.dma_start(tri[:], triM[:])
                t1s = p2c.tile([128, 128], bf16, tag="t1s")
                nc.sync.dma_start(t1s[:], ones_sq[:])

                # flat list of k-chunk-pair units across all (h, q-block)
                units = []
                for h in range(NH):
                    for jb4 in range(NQB):
                        nkc = (QB // 128) * (jb4 + 1)
                        for kp in range(nkc // 2):
                            units.append((h, jb4, kp, nkc))

                blocks = {}   # (h, jb4) -> dict with O, R, A-tiles

                def emit_scores(unit):
                    """S-pair matmuls + exp (+ diag mask) for one unit."""
                    h, jb4, kp, nkc = unit
                    kc0 = 2 * kp
                    kT = qkT[NH + h]
                    qT_b = qkT[h][jb4]
                    Sc = psS.tile([128, 2 * QB], f32, tag="S", name="S")
                    md = kc0 - (QB // 128) * jb4
                    if md < 0:  # fully below the diagonal: full pair
                        for i in range(2):
                            kc = kc0 + i
                            nc.tensor.matmul(
                                Sc[:, i * QB:(i + 1) * QB],
                                kT[kc // 4][:, (kc % 4) * 128:(kc % 4 + 1) * 128],
                                qT_b[:], start=True, stop=True)
                        A = p2.tile([128, 2 * QB], bf16, tag="A", name="A")
                        nc.scalar.activation(A[:], Sc[:], EXP, scale=SCALE)
                    else:  # diagonal pair: trim to live q-range per chunk
                        A = p2.tile([128, 2 * QB], bf16, tag="A", name="A")
                        for i in range(2):
                            kc = kc0 + i
                            off = (md + i) * 128
                            nc.tensor.matmul(
                                Sc[:, i * QB + off:(i + 1) * QB],
                                kT[kc // 4][:, (kc % 4) * 128:(kc % 4 + 1) * 128],
                                qT_b[:, off:], start=True, stop=True)
                            nc.scalar.activation(
                                A[:, i * QB + off:(i + 1) * QB],
                                Sc[:, i * QB + off:(i + 1) * QB],
                                EXP, scale=SCALE)
                            # causal triangle on the leading 128 q columns
                            nc.vector.tensor_tensor(
                                out=A[:, i * QB + off:i * QB + off + 128],
                                in0=A[:, i * QB + off:i * QB + off + 128],
                                in1=tri[:], op=MULT)
                    blocks[(h, jb4)].setdefault("A", {})[kp] = A

                def emit_av(unit):
                    """AV matmuls for one unit (two pairs behind); the rowsum
                    matmuls batch at block end — by then they have no pending
                    exp dependency, so they hide the next block's ramp."""
                    h, jb4, kp, nkc = unit
                    blk = blocks[(h, jb4)]
                    if kp == 0:  # allocate accumulators at first use
                        blk["O"] = psO.tile([128, QB], f32, tag="O", name="O")
                        blk["R"] = psR.tile([128, QB], f32, tag="R", name="R")
                    A = blk["A"][kp]
                    kc0 = 2 * kp
                    md = kc0 - (QB // 128) * jb4
                    for i in range(2):
                        kc = kc0 + i
                        off = max(0, (md + i) * 128)
                        nc.tensor.matmul(
                            blk["O"][:, off:],
                            v_t[kc][:, h * D:(h + 1) * D],
                            A[:, i * QB + off:(i + 1) * QB],
                            start=(kc == 0), stop=(kc == nkc - 1),
                            skip_group_check=True)
                    if kp == nkc // 2 - 1:  # block finished: rowsums, norm
                        for kp2 in range(nkc // 2):
                            A2 = blk["A"].pop(kp2)
                            md2 = 2 * kp2 - (QB // 128) * jb4
                            for i in range(2):
                                kc = 2 * kp2 + i
                                off = max(0, (md2 + i) * 128)
                                nc.tensor.matmul(
                                    blk["R"][:, off:], t1s[:],
                                    A2[:, i * QB + off:(i + 1) * QB],
                                    start=(kc == 0), stop=(kc == nkc - 1),
                                    skip_group_check=True)
                        qsl = slice(jb4 * QB, (jb4 + 1) * QB)
                        rc = p2r.tile([128, QB], f32, tag="rc", name="rc")
                        nc.vector.reciprocal_approx_fast(rc[:], blk["R"][:])
                        nc.vector.tensor_tensor(
                            out=outT[h][:, qsl], in0=blk["O"][:], in1=rc[:],
                            op=MULT)

                DEPTH = 2
                for idx, unit in enumerate(units):
                    h, jb4, kp, nkc = unit
                    if kp == 0:
                        blocks[(h, jb4)] = {}
                    emit_scores(unit)
                    if idx >= DEPTH:
                        emit_av(units[idx - DEPTH])
                for idx in range(len(units) - DEPTH, len(units)):
                    emit_av(units[idx])

              # ---- phase 3: out projection (partial) ----
              with tc.tile_pool(name="p3", bufs=4) as p3, \
                   tc.tile_pool(name="ps3", bufs=4, space="PSUM") as ps3:
                  for tch in range(S // 128):
                      for cbp in range(HID // 1024):
                          ys = p3.tile([128, 1024], bf16, tag="ys", name="ys")
                          for cb2 in range(2):
                              cb = cbp * 2 + cb2
                              P3 = ps3.tile([128, 512], f32, tag="P3", name="P3")
                              for h in range(NH):
                                  nc.tensor.matmul(
                                      P3[:],
                                      outT[h][:, tch * 128:(tch + 1) * 128],
                                      wot[h][:, cb * 512:(cb + 1) * 512],
                                      start=(h == 0), stop=(h == NH - 1))
                              dst = ys[:, cb2 * 512:(cb2 + 1) * 512]
                              if cb % 2 == 0:
                                  nc.vector.tensor_copy(dst, P3[:])
                              else:
                                  nc.scalar.copy(dst, P3[:])
                          nc.sync.dma_start(
                              y[tch * 128:(tch + 1) * 128,
                                cbp * 1024:(cbp + 1) * 1024], ys[:])

    nc.compile()
    return nc


def _host_inputs(x, w_qkv, w_out):
    """Build the 8 per-core input maps."""
    bf16 = ml_dtypes.bfloat16
    # RoPE tables, transposed ([d, t]) with the rotate-half sign folded in.
    inv_freq = 1.0 / (BASE ** (np.arange(0, D, 2, dtype=np.float64) / D))
    pos = np.arange(S, dtype=np.float64)
    freqs = np.outer(inv_freq, pos)           # [64, S]
    cos_h = np.cos(freqs).astype(np.float32)
    sin_h = np.sin(freqs).astype(np.float32)
    cosT = np.concatenate([cos_h, cos_h], 0)  # [128, S]
    sinS = np.concatenate([-sin_h, sin_h], 0)  # signed sin

    # 128x128 causal triangle ([k-part, q-free]): live where q >= k
    kp = np.arange(128)[:, None]
    qf = np.arange(128)[None, :]
    triM = (qf >= kp).astype(bf16)

    w3 = np.asarray(w_qkv, np.float32).reshape(HID, 3, H, D)
    wo_full = np.asarray(w_out, np.float32).reshape(H, D, HID)
    x = np.asarray(x, np.float32)
    xT_b = [np.ascontiguousarray(x[b].T).astype(bf16) for b in range(B)]

    shared = {
        "cosT": cosT, "sinS": sinS, "triM": triM,
        "ones_sq": np.ones((128, 128), bf16),
    }
    in_maps = []
    for c in range(N_CORES):
        b, hg = c // 4, c % 4
        heads = slice(4 * hg, 4 * hg + 4)
        wqk = np.ascontiguousarray(
            w3[:, 0:2, heads, :].reshape(HID, 2 * NH * D)).astype(bf16)
        wv = np.ascontiguousarray(
            w3[:, 2, heads, :].reshape(HID, NH * D)).astype(bf16)
        wo_c = np.ascontiguousarray(
            wo_full[heads].reshape(NH * D, HID)).astype(bf16)
        in_maps.append({
            "xT": xT_b[b],
            "wqk": wqk, "wv": wv, "wo": wo_c, **shared,
        })
    return in_maps


def kernel(x, w_qkv, w_out):
    from concourse.bass_utils import run_bass_kernel_spmd

    if "nc" not in _cache:
        _cache["nc"] = _build()
    nc = _cache["nc"]
    in_maps = _host_inputs(x, w_qkv, w_out)
    res = run_bass_kernel_spmd(nc, in_maps, core_ids=list(range(N_CORES)))
    out = np.zeros((B, S, HID), np.float32)
    for c in range(N_CORES):
        out[c // 4] += res.results[c]["y"].astype(np.float32)
    return out
